# revision 9
# baseline (speedup 1.0000x reference)
"""Trainium2 Bass kernel for nn_InvariantMapping (topk_masking) — v5.

Math: score[b,n] = px.G.py with G_dd' = sum_c fx_d fy_d', px/py the
normalized channel means. Softmax is monotonic, so top-k needs only raw
scores, and the final gather output is exact fp32 values from the host
arrays. Only the RANKING comes from the device; the host re-scores the
top-NCAND candidates exactly in fp64 (validated margin: worst true-top-8
approx rank is 8 out of 512 candidates with bf16 products).

v5 design (vs v2's int8+ACT-cast pipeline):
 - host ships bf16 directly (layout/dtype staging only) -> no ACT cast
   wall (ScalarE runs 1x/elem; casting 50M elems would cost ~330us)
 - DVE computes the 9 Gram product planes in bf16 (2x_1P mode, the only
   fast elementwise path on TRN2) -> ~330us/core, the pacing engine
 - PE reduces all 15 comps over channels with a ones[128,32] stationary,
   rotating the four 32-col tile_position groups per comp so independent
   column-tiles stream concurrently (HW-measured 2.4-3x for col tiling)
 - ScalarE evicts fp32 PSUM -> bf16 SBUF; DMA ships 15 comps/point home
 - host: combine comps -> approx score -> top-512 candidates -> exact
   fp64 rescore -> gather

Sharding: data-parallel over batch, 2 batches per core on 8 cores.
"""
import sys

sys.path.insert(0, "/opt/trn_rl_repo")

import numpy as np
import ml_dtypes

B, C, D, NPTS = 16, 256, 3, 16384
NCORES = 8
BPC = B // NCORES
NT_IO = 2048   # bf16 DMA tile: 4KB contiguous per (c,d) row
NPROD = 1024   # DVE product instruction granularity
NSUB = 512     # PSUM chunk (one fp32 bank)
NCHUNK = NPTS // NSUB
EPS = 1e-6
NCAND = 512    # candidates per batch for the exact host rescore

_CACHE = {}


def _build_nc(bpc=BPC, npts=NPTS, nt_io=NT_IO, nsub=NSUB):
    import concourse.bacc as bacc
    import concourse.bass as bass
    import concourse.mybir as mybir
    import concourse.tile as tile

    f32 = mybir.dt.float32
    bf16 = mybir.dt.bfloat16
    nchunk = npts // nsub

    nc = bacc.Bacc()
    xs = nc.dram_tensor("xs", [bpc, C, D, npts], bf16, kind="ExternalInput")
    fp8 = mybir.dt.float8e4
    ys = nc.dram_tensor("ys", [bpc, C, D, npts], fp8, kind="ExternalInput")
    # comps[b, chunk, q, j, n']: comp k = 4*j + q of point chunk*nsub + n'
    comps = nc.dram_tensor(
        "comps", [bpc, nchunk, 4, 4, nsub], bf16, kind="ExternalOutput"
    )

    with tile.TileContext(nc) as tc:
        with (
            tc.tile_pool(name="io", bufs=2) as iop,
            tc.tile_pool(name="prod", bufs=2) as prodp,
            tc.tile_pool(name="onesp", bufs=1) as onesp,
            tc.tile_pool(name="ps0", bufs=2, space="PSUM") as ps0,
            tc.tile_pool(name="ps1", bufs=2, space="PSUM") as ps1,
            tc.tile_pool(name="ps2", bufs=2, space="PSUM") as ps2,
            tc.tile_pool(name="ps3", bufs=2, space="PSUM") as ps3,
            tc.tile_pool(name="stage", bufs=2) as stagep,
        ):
            psp = [ps0, ps1, ps2, ps3]
            ones = onesp.tile([128, 32], bf16)
            nc.vector.memset(ones, 1.0)

            pending = None

            def _emit_products(pend):
                # product matmuls + eviction for a chunk issued one chunk
                # late, so the DVE product instructions have a full chunk of
                # slack before the PE needs them
                pr_, banks01_, p0_, b_, ts_ = pend
                banks23 = [
                    psp[j].tile([128, nsub], f32, tag="bk", name=f"bank{j}")
                    for j in (2, 3)
                ]
                banks_ = banks01_ + banks23
                for k in range(6, 15):
                    j, q = k // 4, k % 4
                    out = banks_[j][32 * q : 32 * q + 32, :]
                    m = k - 6
                    for g in range(2):
                        rhs = pr_[(g, m // 3)][:, m % 3, p0_ : p0_ + nsub]
                        nc.tensor.matmul(
                            out, ones, rhs,
                            start=(g == 0), stop=(g == 1),
                            tile_position=(0, 32 * q),
                        )
                # evict PSUM -> SBUF bf16 on ScalarE (bank 3 has only 3
                # comps; quad 3 is never written)
                st = stagep.tile([128, 4 * nsub], bf16, tag="st")
                for j in range(4):
                    np_used = 128 if j < 3 else 96
                    nc.scalar.copy(
                        out=st[:np_used, nsub * j : nsub * (j + 1)],
                        in_=banks_[j][:np_used],
                    )
                strided = bass.AP(
                    tensor=st.tensor,
                    offset=st.offset,
                    ap=[[32 * st.ap[0][0], 4]] + list(st.ap[1:]),
                )
                nc.sync.dma_start(out=comps[b_, ts_], in_=strided)

            for b in range(bpc):
                for t in range(npts // nt_io):
                    n0 = nt_io * t
                    xh, yh = [], []
                    for g in range(2):
                        c0 = 128 * g
                        xg = iop.tile([128, D, nt_io], bf16, tag=f"xi{g}")
                        y8 = iop.tile([128, D, nt_io], fp8, tag=f"y8{g}")
                        nc.sync.dma_start(
                            out=xg, in_=xs[b, c0 : c0 + 128, :, n0 : n0 + nt_io]
                        )
                        nc.sync.dma_start(
                            out=y8, in_=ys[b, c0 : c0 + 128, :, n0 : n0 + nt_io]
                        )
                        yg = iop.tile([128, D, nt_io], bf16, tag=f"yi{g}")
                        nc.scalar.copy(out=yg, in_=y8)
                        xh.append(xg)
                        yh.append(yg)

                    for h in range(nt_io // NPROD):
                        m0 = NPROD * h
                        # 9 Gram product planes per c-group in ONE DVE instr:
                        # pr[g][:, 3*d+dp, :] = x_d * y_dp (x bcast over dp,
                        # y bcast over d)
                        pr = {}
                        for g in range(2):
                            for d in range(D):
                                p = prodp.tile(
                                    [128, D, NPROD], bf16, tag=f"pr{g}{d}"
                                )
                                nc.vector.tensor_mul(
                                    p,
                                    xh[g][
                                        :, d : d + 1, m0 : m0 + NPROD
                                    ].to_broadcast([128, D, NPROD]),
                                    yh[g][:, :, m0 : m0 + NPROD],
                                )
                                pr[(g, d)] = p

                        for u in range(NPROD // nsub):
                            s0 = m0 + nsub * u
                            p0 = nsub * u
                            ts = (t * nt_io + s0) // nsub
                            # means for THIS chunk (deps: DMA only) -> the PE
                            # always has dep-free work while DVE computes the
                            # products, which are consumed one chunk later
                            banks01 = [
                                psp[j].tile(
                                    [128, nsub], f32, tag="bk", name=f"bank{j}"
                                )
                                for j in range(2)
                            ]
                            for k in range(6):
                                j, q = k // 4, k % 4
                                out = banks01[j][32 * q : 32 * q + 32, :]
                                for g in range(2):
                                    rhs = (
                                        xh[g][:, k, s0 : s0 + nsub]
                                        if k < 3
                                        else yh[g][:, k - 3, s0 : s0 + nsub]
                                    )
                                    nc.tensor.matmul(
                                        out, ones, rhs,
                                        start=(g == 0), stop=(g == 1),
                                        tile_position=(0, 32 * q),
                                    )
                            if pending is not None:
                                _emit_products(pending)
                            pending = (pr, banks01, p0, b, ts)
                        # (end u loop)
                    # (end h loop)
                # (end t loop)
            # flush the last chunk's products
            if pending is not None:
                _emit_products(pending)
                pending = None
    nc.finalize()
    return nc


def _get_nc():
    if "nc" not in _CACHE:
        _CACHE["nc"] = _build_nc()
    return _CACHE["nc"]


def _stage(fx, fy):
    """x: fp32 -> bf16 (RNE bit twiddle). y: fp32 -> fp8e4m3, scaled so
    amax -> ~240 (the e4m3 max). The uniform y-scale cancels in the
    normalized approx score, and the exact host rescore uses the raw
    fp32 arrays."""
    u = np.ascontiguousarray(fx, np.float32).view(np.uint32)
    r = ((u + 0x7FFF + ((u >> 16) & 1)) >> 16).astype(np.uint16)
    xb = r.view(ml_dtypes.bfloat16).reshape(fx.shape)
    s = 240.0 * 0.98 / max(float(fy.max()), -float(fy.min()), 1e-30)
    yq = (np.asarray(fy, np.float32) * s).astype(ml_dtypes.float8_e4m3)
    return xb, yq


def _run_device(xb, yb, trace=False):
    from concourse.bass_utils import run_bass_kernel_spmd

    nc = _get_nc()
    in_maps = []
    for i in range(NCORES):
        sl = slice(BPC * i, BPC * (i + 1))
        in_maps.append({"xs": xb[sl], "ys": yb[sl]})
    res = run_bass_kernel_spmd(nc, in_maps, core_ids=list(range(NCORES)), trace=trace)
    out = np.stack([r["comps"] for r in res.results])  # [8, BPC, NCHUNK, 4, 4, NSUB]
    return out, res


def _approx_scores(out):
    # out: [8, BPC, NCHUNK, 4(q), 4(j), NSUB] -> comp k=4j+q -> [B, 16, NPTS]
    a = np.asarray(out, np.float64).reshape(B, NCHUNK, 4, 4, NSUB)
    a = a.transpose(0, 3, 2, 1, 4).reshape(B, 16, NPTS)  # [b, k=4j+q, n]
    Sx = a[:, 0:3]
    Sy = a[:, 3:6]
    G = a[:, 6:15].reshape(B, 3, 3, NPTS)
    nx = np.sqrt((Sx**2).sum(1, keepdims=True)) + EPS
    ny = np.sqrt((Sy**2).sum(1, keepdims=True)) + EPS
    px = Sx / nx
    py = Sy / ny
    return np.einsum("bdn,bden,ben->bn", px, G, py)


def _exact_topk(fx, fy, cand, kk):
    # exact fp64 rescore of candidate columns; returns [B, kk] indices in
    # jax.lax.top_k order (desc value, ties -> lower index)
    idx = np.empty((B, kk), np.int64)
    for b in range(B):
        cols = np.sort(cand[b])
        fxc = fx[b][:, :, cols].astype(np.float64)  # [C, D, m]
        fyc = fy[b][:, :, cols].astype(np.float64)
        mx = fxc.mean(0)  # [D, m]
        my = fyc.mean(0)
        px = mx / (np.sqrt((mx**2).sum(0, keepdims=True)) + EPS)
        py = my / (np.sqrt((my**2).sum(0, keepdims=True)) + EPS)
        phix = np.einsum("cdm,dm->mc", fxc, px)
        phiy = np.einsum("cdm,dm->mc", fyc, py)
        s = np.einsum("mc,mc->m", phix, phiy)
        order = np.argsort(-s, kind="stable")[:kk]
        idx[b] = cols[order]
    return idx


def kernel(fx, fy, topk):
    fx = np.asarray(fx, dtype=np.float32)
    fy = np.asarray(fy, dtype=np.float32)
    kk = B // int(topk)

    xb, yb = _stage(fx, fy)
    out, _ = _run_device(xb, yb)
    score = _approx_scores(out)

    ncand = max(NCAND, kk)
    cand = np.argpartition(-score, ncand - 1, axis=1)[:, :ncand]
    idx = _exact_topk(fx, fy, cand, kk).astype(np.int32)

    idxe = idx[:, None, None, :]
    fx_sel = np.take_along_axis(fx, idxe, axis=3)
    fy_sel = np.take_along_axis(fy, idxe, axis=3)
    return (fx_sel, fy_sel)


# revision 14
# speedup vs baseline: 1.0267x; 1.0267x over previous
"""Trainium2 Bass kernel for nn_InvariantMapping (topk_masking) — v6.

Math: score[b,n] = px.G.py with G_dd' = sum_c fx_d fy_d', px/py the
normalized channel means. Softmax is monotonic, so top-k needs only raw
scores, and the final gather output is exact fp32 values from the host
arrays. Only the RANKING comes from the device; the host re-scores the
top-NCAND candidates exactly in fp64 (validated on the actual data:
worst true-top-8 approx rank is 11 of the 512 candidates under this
pipeline's bf16-x / fp8-y quantization — a 46x margin).

Design, from HW-measured engine rates (per core: DMA ~210-235 GB/s, DVE
tensor_tensor bf16 ~0.75 cyc/elem, ScalarE 1x/elem, PE ones-matmul
~154-233 ns per N=512 stream with rotated column-tiles; changing-weight
matmuls cost 350-570 ns each, which kills all "Gram via PE stationary"
schemes, and GPSIMD assist loses via the shared SBUF port):
 - host ships x as bf16 and y as fp8e4m3 (75.5 MB/core vs 100.6 all-bf16;
   DMA ~360us), pure dtype staging — the y scale cancels in the score
 - ScalarE casts y fp8->bf16 on device (~170us) + evicts PSUM (~150us)
 - DVE computes the 9 Gram product planes in bf16 2x_1P (~470us/core;
   measured — THE pacing engine; no other TRN2 engine can do elementwise
   two-tensor multiplies at a useful rate)
 - PE reduces all 15 comps (6 raw means + 9 products) over the two
   128-channel groups with a resident ones[128,32] stationary, rotating
   the four 32-col tile_position groups per comp so consecutive
   accumulation pairs hit different column-tiles (~300-450us, hidden)
 - DMA ships 15 comps/point home as bf16 (2 MB/core)
 - host: combine comps -> approx score -> top-512 candidates -> exact
   fp64 rescore -> gather

Sharding: data-parallel over batch, 2 batches per core on 8 cores.
"""
import sys

sys.path.insert(0, "/opt/trn_rl_repo")

import numpy as np
import ml_dtypes

B, C, D, NPTS = 16, 256, 3, 16384
NCORES = 8
BPC = B // NCORES
NT_IO = 2048   # bf16 DMA tile: 4KB contiguous per (c,d) row
NPROD = 1024   # DVE product instruction granularity
NSUB = 512     # PSUM chunk (one fp32 bank)
NCHUNK = NPTS // NSUB
EPS = 1e-6
NCAND = 512    # candidates per batch for the exact host rescore

_CACHE = {}


def _build_nc(bpc=BPC, npts=NPTS, nt_io=NT_IO, nsub=NSUB):
    import concourse.bacc as bacc
    import concourse.bass as bass
    import concourse.mybir as mybir
    import concourse.tile as tile

    f32 = mybir.dt.float32
    bf16 = mybir.dt.bfloat16
    nchunk = npts // nsub

    nc = bacc.Bacc()
    xs = nc.dram_tensor("xs", [bpc, C, D, npts], bf16, kind="ExternalInput")
    fp8 = mybir.dt.float8e4
    ys = nc.dram_tensor("ys", [bpc, C, D, npts], fp8, kind="ExternalInput")
    # comps[b, chunk, q, j, n']: comp k = 4*j + q of point chunk*nsub + n'
    comps = nc.dram_tensor(
        "comps", [bpc, nchunk, 4, 4, nsub], bf16, kind="ExternalOutput"
    )

    with tile.TileContext(nc) as tc:
        with (
            tc.tile_pool(name="io", bufs=2) as iop,
            tc.tile_pool(name="prod", bufs=2) as prodp,
            tc.tile_pool(name="ycast", bufs=3) as ycastp,
            tc.tile_pool(name="onesp", bufs=1) as onesp,
            tc.tile_pool(name="ps0", bufs=2, space="PSUM") as ps0,
            tc.tile_pool(name="ps1", bufs=2, space="PSUM") as ps1,
            tc.tile_pool(name="ps2", bufs=2, space="PSUM") as ps2,
            tc.tile_pool(name="ps3", bufs=2, space="PSUM") as ps3,
            tc.tile_pool(name="stage", bufs=2) as stagep,
        ):
            psp = [ps0, ps1, ps2, ps3]
            ones = onesp.tile([128, 32], bf16)
            nc.vector.memset(ones, 1.0)

            for b in range(bpc):
                for t in range(npts // nt_io):
                    n0 = nt_io * t
                    xh, yh = [], []
                    for g in range(2):
                        c0 = 128 * g
                        xg = iop.tile([128, D, nt_io], bf16, tag=f"xi{g}")
                        y8 = iop.tile([128, D, nt_io], fp8, tag=f"y8{g}")
                        nc.sync.dma_start(
                            out=xg, in_=xs[b, c0 : c0 + 128, :, n0 : n0 + nt_io]
                        )
                        nc.sync.dma_start(
                            out=y8, in_=ys[b, c0 : c0 + 128, :, n0 : n0 + nt_io]
                        )
                        xh.append(xg)
                        yh.append(y8)

                    for h in range(nt_io // NPROD):
                        m0 = NPROD * h
                        # cast this half of y to bf16 (ScalarE), then the 9
                        # Gram product planes per c-group, 3 per DVE instr:
                        # pr[(g,d)][:, dp, :] = x_d * y_dp (x_d broadcast)
                        yb = []
                        for g in range(2):
                            yc = ycastp.tile([128, D, NPROD], bf16, tag=f"yc{g}")
                            nc.scalar.copy(
                                out=yc, in_=yh[g][:, :, m0 : m0 + NPROD]
                            )
                            yb.append(yc)
                        pr = {}
                        for g in range(2):
                            for d in range(D):
                                p = prodp.tile(
                                    [128, D, NPROD], bf16, tag=f"pr{g}{d}"
                                )
                                nc.vector.tensor_mul(
                                    p,
                                    xh[g][
                                        :, d : d + 1, m0 : m0 + NPROD
                                    ].to_broadcast([128, D, NPROD]),
                                    yb[g],
                                )
                                pr[(g, d)] = p

                        for u in range(NPROD // nsub):
                            s0 = m0 + nsub * u
                            p0 = nsub * u
                            ts = (t * nt_io + s0) // nsub
                            banks = [
                                psp[j].tile(
                                    [128, nsub], f32, tag="bk", name=f"bank{j}"
                                )
                                for j in range(4)
                            ]
                            for k in range(15):
                                j, q = k // 4, k % 4
                                out = banks[j][32 * q : 32 * q + 32, :]
                                for g in range(2):
                                    if k < 3:
                                        rhs = xh[g][:, k, s0 : s0 + nsub]
                                    elif k < 6:
                                        rhs = yb[g][:, k - 3, p0 : p0 + nsub]
                                    else:
                                        m = k - 6
                                        rhs = pr[(g, m // 3)][
                                            :, m % 3, p0 : p0 + nsub
                                        ]
                                    nc.tensor.matmul(
                                        out,
                                        ones,
                                        rhs,
                                        start=(g == 0),
                                        stop=(g == 1),
                                        tile_position=(0, 32 * q),
                                    )
                            st = stagep.tile([128, 4 * nsub], bf16, tag="st")
                            for j in range(4):
                                np_used = 128 if j < 3 else 96
                                nc.scalar.copy(
                                    out=st[:np_used, nsub * j : nsub * (j + 1)],
                                    in_=banks[j][:np_used],
                                )
                            strided = bass.AP(
                                tensor=st.tensor,
                                offset=st.offset,
                                ap=[[32 * st.ap[0][0], 4]] + list(st.ap[1:]),
                            )
                            nc.sync.dma_start(out=comps[b, ts], in_=strided)
    nc.finalize()
    return nc


def _get_nc():
    if "nc" not in _CACHE:
        _CACHE["nc"] = _build_nc()
    return _CACHE["nc"]


def _stage(fx, fy):
    """x: fp32 -> bf16 (RNE bit twiddle). y: fp32 -> fp8e4m3, scaled so
    amax -> ~240 (the e4m3 max). The uniform y-scale cancels in the
    normalized approx score, and the exact host rescore uses the raw
    fp32 arrays."""
    u = np.ascontiguousarray(fx, np.float32).view(np.uint32)
    r = ((u + 0x7FFF + ((u >> 16) & 1)) >> 16).astype(np.uint16)
    xb = r.view(ml_dtypes.bfloat16).reshape(fx.shape)
    s = 240.0 * 0.98 / max(float(fy.max()), -float(fy.min()), 1e-30)
    yq = (np.asarray(fy, np.float32) * s).astype(ml_dtypes.float8_e4m3)
    return xb, yq


def _run_device(xb, yb, trace=False):
    from concourse.bass_utils import run_bass_kernel_spmd

    nc = _get_nc()
    in_maps = []
    for i in range(NCORES):
        sl = slice(BPC * i, BPC * (i + 1))
        in_maps.append({"xs": xb[sl], "ys": yb[sl]})
    res = run_bass_kernel_spmd(nc, in_maps, core_ids=list(range(NCORES)), trace=trace)
    out = np.stack([r["comps"] for r in res.results])  # [8, BPC, NCHUNK, 4, 4, NSUB]
    return out, res


def _approx_scores(out):
    # out: [8, BPC, NCHUNK, 4(q), 4(j), NSUB] -> comp k=4j+q -> [B, 16, NPTS]
    a = np.asarray(out, np.float64).reshape(B, NCHUNK, 4, 4, NSUB)
    a = a.transpose(0, 3, 2, 1, 4).reshape(B, 16, NPTS)  # [b, k=4j+q, n]
    Sx = a[:, 0:3]
    Sy = a[:, 3:6]
    G = a[:, 6:15].reshape(B, 3, 3, NPTS)
    nx = np.sqrt((Sx**2).sum(1, keepdims=True)) + EPS
    ny = np.sqrt((Sy**2).sum(1, keepdims=True)) + EPS
    px = Sx / nx
    py = Sy / ny
    return np.einsum("bdn,bden,ben->bn", px, G, py)


def _exact_topk(fx, fy, cand, kk):
    # exact fp64 rescore of candidate columns; returns [B, kk] indices in
    # jax.lax.top_k order (desc value, ties -> lower index)
    idx = np.empty((B, kk), np.int64)
    for b in range(B):
        cols = np.sort(cand[b])
        fxc = fx[b][:, :, cols].astype(np.float64)  # [C, D, m]
        fyc = fy[b][:, :, cols].astype(np.float64)
        mx = fxc.mean(0)  # [D, m]
        my = fyc.mean(0)
        px = mx / (np.sqrt((mx**2).sum(0, keepdims=True)) + EPS)
        py = my / (np.sqrt((my**2).sum(0, keepdims=True)) + EPS)
        phix = np.einsum("cdm,dm->mc", fxc, px)
        phiy = np.einsum("cdm,dm->mc", fyc, py)
        s = np.einsum("mc,mc->m", phix, phiy)
        order = np.argsort(-s, kind="stable")[:kk]
        idx[b] = cols[order]
    return idx


def kernel(fx, fy, topk):
    fx = np.asarray(fx, dtype=np.float32)
    fy = np.asarray(fy, dtype=np.float32)
    kk = B // int(topk)

    xb, yb = _stage(fx, fy)
    out, _ = _run_device(xb, yb)
    score = _approx_scores(out)

    ncand = max(NCAND, kk)
    cand = np.argpartition(-score, ncand - 1, axis=1)[:, :ncand]
    idx = _exact_topk(fx, fy, cand, kk).astype(np.int32)

    idxe = idx[:, None, None, :]
    fx_sel = np.take_along_axis(fx, idxe, axis=3)
    fy_sel = np.take_along_axis(fy, idxe, axis=3)
    return (fx_sel, fy_sel)
